# revision 1
# baseline (speedup 1.0000x reference)
"""Trainium2 Bass kernel for the 81-step LSTM decoder + masked softmax.

Math (per batch row b):
    z_t = x_t @ W_x + h_{t-1} @ W_h + b          (gates i, f, g, o; 100 each)
    i,f,o = sigmoid;  g = identity
    c_t = f*c_{t-1} + i*g;  h_t = o*c_t
    out_t = softmax(where(mask_t, h_t, -inf))

Strategy: data-parallel over batch (4096 -> 8 cores x 512). Each core runs
an identical Bass program on its shard; no collectives.

Device layout is feature-major ("transposed"): the recurrent state h^T is
kept as [101, 512] (hidden-on-partitions, batch-on-free, +1 ones row so the
bias rides in an augmented weight row).  x is fed to the device already
transposed on the host as xT [81, 512e, 512b] so the contraction dim (e)
lands on partitions with zero on-device transposes.  All matmuls run as
float32r (fp32 storage, relaxed-precision PE mode, 1 cyc/row at N>=256).
The only on-device transposes are 4 small PE transposes per step to bring
the masked exp(h) back to batch-major for the row softmax and output DMA.
"""

import sys

if "/opt/trn_rl_repo" not in sys.path:
    sys.path.insert(0, "/opt/trn_rl_repo")

import numpy as np

P = 81       # places / timesteps
H = 100      # LSTM units
E = 512      # encoder feature width
B = 4096     # total batch
NCORES = 8
BS = B // NCORES          # 512 batch rows per core
NB = BS // 128            # 4 batch tiles of 128
NE = E // 128             # 4 feature chunks of 128
MASK_NEG = -1.0e5         # exp(h + MASK_NEG) == 0.0 exactly in fp32
K = 9                     # softmax/exp batching window (81 % 9 == 0)

_PROGRAM = None


def _build_program():
    import concourse.bacc as bacc
    import concourse.bass as bass
    import concourse.mybir as mybir
    from concourse.tile import TileContext
    from concourse.tile_rust import add_dep_helper
    from contextlib import ExitStack

    f32 = mybir.dt.float32
    f32r = mybir.dt.float32r
    bf16 = mybir.dt.bfloat16
    SIG = mybir.ActivationFunctionType.Sigmoid
    EXP = mybir.ActivationFunctionType.Exp
    CPY = mybir.ActivationFunctionType.Copy
    ADD = mybir.AluOpType.add
    X = mybir.AxisListType.X

    nc = bacc.Bacc(None, target_bir_lowering=False)

    # read-only constants packed into one tensor/DMA to use one DMA queue
    # layout along free dim: wx [0:1600], whb [1600:2000], maskb [2000:2081],
    # ident [2081:2209]
    CW = NE * 400
    C_WHB = CW
    C_MB = C_WHB + 400
    C_MB2 = C_MB + P
    C_ID = C_MB2 + P
    C_TOT = C_ID + 128
    xT_d = nc.dram_tensor("xT", [P, E, BS], bf16, kind="ExternalInput")
    wxb_d = nc.dram_tensor("wxb", [128, CW], bf16, kind="ExternalInput")
    consts_d = nc.dram_tensor("consts", [128, C_TOT], f32r, kind="ExternalInput")
    h0T_d = nc.dram_tensor("h0T", [H + 1, BS], f32r, kind="ExternalInput")
    out_d = nc.dram_tensor("out", [BS, P, H], f32, kind="ExternalOutput")

    with ExitStack() as ctx:
        tc = ctx.enter_context(TileContext(nc))
        consts = ctx.enter_context(tc.tile_pool(name="consts", bufs=1))
        xpool = ctx.enter_context(tc.tile_pool(name="xpool", bufs=16))
        gpool = ctx.enter_context(tc.tile_pool(name="gpool", bufs=2))
        opool = ctx.enter_context(tc.tile_pool(name="opool", bufs=8))
        zpool = ctx.enter_context(tc.tile_pool(name="zpool", bufs=6, space="PSUM"))
        epool = ctx.enter_context(tc.tile_pool(name="epool", bufs=2, space="PSUM"))

        csb = consts.tile([128, C_TOT], f32r)
        nc.sync.dma_start(out=csb, in_=consts_d[:, :])
        wxb = consts.tile([128, CW], bf16)
        nc.sync.dma_start(out=wxb, in_=wxb_d[:, :])
        whb = csb[0 : H + 1, C_WHB : C_WHB + 400]
        idn = csb[:, C_ID : C_ID + 128]
        # ring of recurrent-state snapshots; row H holds the constant 1.0 that
        # multiplies the bias row of whb
        hist = [consts.tile([H + 1, BS], f32r, name=f"hist{j}") for j in range(K)]
        # seed the constant-1.0 row of each snapshot from h0T's ones row
        for j in range(K - 1):
            nc.sync.dma_start(out=hist[j][H : H + 1, :], in_=h0T_d[H : H + 1, :])
        nc.sync.dma_start(out=hist[K - 1], in_=h0T_d[:, :])
        cT = consts.tile([H, BS], f32)           # persistent cell state
        nc.vector.memset(cT, 0.0)

        # W column order: i [0:100], f [100:200], g [200:300], o [300:400]
        # zifo bank j: 0 -> i, 1 -> f, 2 -> o ; zg separate (identity gate)
        GSLICE = [(0, 0), (1, 1), (3, 2), (2, None)]  # (w-col-block, zifo bank)

        # ACT instructions that use the LUT (sigmoid/exp) are chained in
        # program order so the scheduler cannot interleave exp and sigmoid
        # arbitrarily -- the activation table reload costs ~1.3us each.
        act_prev = [None]

        def act_ordered(bi):
            if act_prev[0] is not None:
                add_dep_helper(bi.ins, act_prev[0].ins, sync=False, reason="act order")
            act_prev[0] = bi

        def softmax_tail(tau):
            e = gpool.tile([H, BS], f32r, name=f"e_{tau}", tag="e", bufs=10)
            act_ordered(
                nc.scalar.activation(
                    e,
                    hist[tau % K].bitcast(f32)[0:H, :],
                    EXP,
                    bias=csb.bitcast(f32)[0:H, C_MB + tau : C_MB + tau + 1],
                )
            )
            eT = epool.tile([128, NB, 128], f32r, name=f"eT_{tau}", tag="eT")
            for k in range(NB):
                nc.tensor.transpose(
                    eT[:, k, 0:H],
                    e[:, 128 * k : 128 * (k + 1)],
                    idn[0:H, 0:H],
                )
            s = opool.tile([128, NB], f32, name=f"s_{tau}", tag="s")
            nc.vector.tensor_reduce(s, eT.bitcast(f32)[:, :, 0:H], axis=X, op=ADD)
            r = opool.tile([128, NB], f32, name=f"r_{tau}", tag="r")
            nc.vector.reciprocal(r, s)
            for k in range(NB):
                ot = opool.tile([128, H], f32, name=f"ot_{tau}_{k}", tag="ot")
                nc.vector.tensor_scalar_mul(
                    ot, eT.bitcast(f32)[:, k, 0:H], r[:, k : k + 1]
                )
                nc.sync.dma_start(out=out_d[128 * k : 128 * (k + 1), tau, :], in_=ot)

        for t in range(P):
            # ---- stream x_t^T in, feature chunks on partitions (one DMA) ----
            xtile = xpool.tile([128, NE, BS], bf16, name=f"x_{t}", tag="x")
            nc.sync.dma_start(
                out=xtile, in_=xT_d[t].rearrange("(c p) b -> p c b", p=128)
            )

            # ---- z^T per gate (i, f, g, o), four PSUM banks ----
            zg = [None] * 4
            for wcol in (1, 0, 2, 3):
                z = zpool.tile([H, BS], f32, name=f"z_{t}_{wcol}", tag="z")
                for ec in range(NE):
                    nc.tensor.matmul(
                        z,
                        wxb[:, ec * 400 + wcol * H : ec * 400 + (wcol + 1) * H],
                        xtile[:, ec, :],
                        start=(ec == 0),
                        stop=False,
                    )
                nc.tensor.matmul(
                    z,
                    whb[:, wcol * H : (wcol + 1) * H],
                    hist[(t - 1) % K],
                    start=False,
                    stop=True,
                )
                zg[wcol] = z

            # ---- gates (f first: t1 is the head of the DVE chain) ----
            f_s = gpool.tile([H, BS], f32, name=f"f_{t}", tag="f")
            act_ordered(nc.scalar.activation(f_s, zg[1], SIG))
            i_s = gpool.tile([H, BS], f32, name=f"i_{t}", tag="i")
            act_ordered(nc.scalar.activation(i_s, zg[0], SIG))
            o_s = gpool.tile([H, BS], f32, name=f"o_{t}", tag="o")
            act_ordered(nc.scalar.activation(o_s, zg[3], SIG))

            t1 = gpool.tile([H, BS], f32, name=f"t1_{t}", tag="t1")
            nc.vector.tensor_mul(t1, f_s, cT)                  # f * c_{t-1}
            t2 = gpool.tile([H, BS], f32, name=f"t2_{t}", tag="t2")
            nc.vector.tensor_mul(t2, i_s, zg[2])               # i * g
            nc.vector.tensor_add(cT, t1, t2)                   # c_t
            nc.vector.tensor_mul(hist[t % K][0:H, :], o_s, cT)  # h_t

            if t % K == K - 1:
                for tau in range(t - K + 1, t + 1):
                    softmax_tail(tau)

    nc.compile()
    return nc


def _get_program():
    global _PROGRAM
    if _PROGRAM is None:
        _PROGRAM = _build_program()
    return _PROGRAM


def _prep_in_maps(h_enc, h0, W_x, W_h, b, mask):
    h_enc = np.asarray(h_enc, dtype=np.float32)
    h0 = np.asarray(h0, dtype=np.float32)
    W_x = np.asarray(W_x, dtype=np.float32)
    W_h = np.asarray(W_h, dtype=np.float32)
    b = np.asarray(b, dtype=np.float32)
    mask = np.asarray(mask)

    # lhsT layout for the xW matmuls: row p holds W_x[ec*128 + p, :] for the
    # 4 feature chunks side by side -> [128, 4*400]
    wx_sb = W_x.reshape(NE, 128, 400).transpose(1, 0, 2).reshape(128, NE * 400)
    # packed read-only consts: [wx | whb | maskb | ident]
    CW = NE * 400
    C_TOT = CW + 400 + 2 * P + 128
    consts = np.zeros((128, C_TOT), np.float32)
    consts[:, 0:CW] = wx_sb
    consts[0:H, CW : CW + 400] = W_h
    consts[H, CW : CW + 400] = b
    mb = np.where(mask, 0.0, MASK_NEG).astype(np.float32).T
    consts[0:H, CW + 400 : CW + 400 + P] = mb
    consts[0:H, CW + 400 + P : CW + 400 + 2 * P] = -mb
    consts[:, CW + 400 + 2 * P :] = np.eye(128, dtype=np.float32)

    import ml_dtypes

    bf16 = ml_dtypes.bfloat16
    wxb = np.ascontiguousarray(wx_sb).astype(bf16)
    in_maps = []
    xTf = np.empty((P, E, BS), np.float32)
    for c in range(NCORES):
        shard = h_enc[c * BS : (c + 1) * BS]  # [BS, P, E]
        for t in range(P):
            xTf[t] = shard[:, t, :].T
        xT = xTf.astype(bf16)
        h0T = np.ascontiguousarray(
            np.concatenate(
                [h0[c * BS : (c + 1) * BS].T, np.ones((1, BS), np.float32)], axis=0
            )
        )
        in_maps.append({"xT": xT, "wxb": wxb, "consts": consts, "h0T": h0T})
    return in_maps


def run(inputs: dict, trace: bool = False):
    """Run on 8 cores; returns (full_output, exec_time_ns_or_None)."""
    from concourse.bass_utils import run_bass_kernel_spmd

    nc = _get_program()
    in_maps = _prep_in_maps(**inputs)
    res = run_bass_kernel_spmd(
        nc, in_maps, core_ids=list(range(NCORES)), trace=trace
    )
    out = np.concatenate([r["out"] for r in res.results], axis=0)
    return out, res.exec_time_ns


def kernel(**inputs) -> np.ndarray:
    out, _ = run(inputs, trace=False)
    return out

